# revision 1
# baseline (speedup 1.0000x reference)
"""Trainium2 Bass kernel for batched windowed DFT (STFT-as-GEMM).

Problem: for each batch row of x (8, 262144), reflect-pad by 1024, frame into
513 overlapping windows (len 2048, hop 512), and multiply by dense Hann-windowed
sin/cos DFT matrices (2048x2048):  real = wcos @ frames^T, out = (real, -imag).

Strategy (one batch per NeuronCore, 8 cores):
  * Hermitian symmetry of the real-input DFT: compute bins k=0..1151 only;
    rows 1152..2047 are mirrors (real: copy, imag: sign flip) of rows 896..1.
  * Even/odd fold of the contraction: the Hann-windowed basis obeys
    w[k, 2048-n] = +/- w[k, n], so contract only n=1..1023 against folded
    frames S+- = f[n] -+ f[2048-n] (computed on-chip by the vector engine),
    plus a rank-1 edge term for n=1024 (folded into the PSUM accumulation as
    an extra matmul).  Halves both FLOPs and weight traffic.
  * fp16 matmuls (11-bit mantissa, ~2.8e-4 rel err end-to-end) at full PE
    rate with pipelined weight loads; fp32 PSUM accumulation and outputs.
  * Host side does only data marshalling: reflect pad, layout interleave
    (phase-split so every on-chip access is contiguous), weight transpose +
    fp16 cast, and final gather/flip assembly of the mirrored halves.
"""
import numpy as np

import concourse.bacc as bacc
import concourse.mybir as mybir
import concourse.tile as tile

F32 = mybir.dt.float32
F16 = mybir.dt.float16
T = 513          # frames
TP = 514         # padded (junk col 513) so the tail matmul has even free dim
PH = 520         # per-phase column pitch of the phase-split signal layout
NKT = 9          # k tiles computed directly: k = 0..1151
NA = 8           # folded contraction chunks of 128 (n = 0..1023)
WCOLS = 2 * NKT * 128   # sin block | cos block
N_CORES = 8
L_PAD = 264192   # 262144 + 2*1024


def build_nc(reps=1):
    nc = bacc.Bacc("TRN2", target_bir_lowering=False, debug=False,
                   num_devices=N_CORES)
    xa_d = nc.dram_tensor("xa", [128, 4 * PH], F32, kind="ExternalInput")
    xr_d = nc.dram_tensor("xr", [128, 4 * PH], F32, kind="ExternalInput")
    wt_d = nc.dram_tensor("wt", [NKT * 128, 2048], F16, kind="ExternalInput")
    sgn_d = nc.dram_tensor("sgn", [128, 128], F16, kind="ExternalInput")
    outR_d = nc.dram_tensor("outR", [NKT * 128, T], F32, kind="ExternalOutput")
    outI_d = nc.dram_tensor("outI", [NKT * 128, T], F32, kind="ExternalOutput")
    outIm_d = nc.dram_tensor("outIm", [896, T], F32, kind="ExternalOutput")

    with tile.TileContext(nc) as tc:
        with (
            tc.tile_pool(name="xin", bufs=1) as xin,
            tc.tile_pool(name="wts", bufs=1) as wts,
            tc.tile_pool(name="sbf", bufs=1) as sbf,
            tc.tile_pool(name="stag", bufs=1) as stag,
            tc.tile_pool(name="ps", bufs=4, space="PSUM") as ps,
        ):
            XA = xin.tile([128, 4 * PH], F32, tag="XA")
            XR = xin.tile([128, 4 * PH], F32, tag="XR")
            SG = xin.tile([128, 128], F16, tag="SG")
            W = wts.tile([128, NKT * 2048], F16, tag="W")
            Sm = sbf.tile([128, NA * TP], F16, tag="Sm")
            Sp = sbf.tile([128, NA * TP], F16, tag="Sp")
            Vv = sbf.tile([128, TP], F16, tag="Vv")
            stC = stag.tile([128, NKT * T], F32, tag="stC")
            stN = stag.tile([128, NKT * T], F32, tag="stN")
            stP = stag.tile([128, NA * T], F32, tag="stP")

            for _rep in range(reps):
                nc.sync.dma_start(XA[:], xa_d.ap())
                nc.sync.dma_start(XR[:], xr_d.ap())
                nc.sync.dma_start(SG[:], sgn_d.ap())
                for kt in range(NKT):
                    nc.sync.dma_start(W[:, kt * 2048:(kt + 1) * 2048],
                                      wt_d.ap()[kt * 128:(kt + 1) * 128, :])

                # folds: S-+[a][p, t] = f[128a+p] -+ f[2048-(128a+p)] per frame t
                for a in range(NA):
                    xa_s = XA[:, (a % 4) * PH + a // 4:(a % 4) * PH + a // 4 + TP]
                    ph = (15 - a) % 4
                    off = (15 - a) // 4
                    xr_s = XR[:, ph * PH + off:ph * PH + off + TP]
                    nc.vector.tensor_sub(Sm[:, a * TP:a * TP + TP], xa_s, xr_s)
                    nc.vector.tensor_add(Sp[:, a * TP:a * TP + TP], xa_s, xr_s)
                # edge rhs: row p = xp[512t + 1024 + p]; only row 0 is weighted
                nc.vector.tensor_copy(Vv[:], XA[:, 2:2 + TP])

                for kt in range(NKT):
                    for ch in range(2):  # 0 = sin (imag), 1 = cos (real)
                        acc = ps.tile([128, 1024], F32, tag="acc")
                        S = Sp if ch else Sm
                        last = NA - 1 if ch == 0 else NA
                        for a in range(NA):
                            wo = kt * 2048 + a * 256 + ch * 128
                            lhsT = W[:, wo:wo + 128]
                            nc.tensor.matmul(acc[:, 0:512], lhsT,
                                             S[:, a * TP:a * TP + 512],
                                             start=(a == 0), stop=(a == last))
                            nc.tensor.matmul(acc[:, 512:514], lhsT,
                                             S[:, a * TP + 512:a * TP + 514],
                                             start=(a == 0), stop=(a == last))
                        if ch == 1:
                            # n=1024 edge: acc[p, t] += (-1)^p * xp[512t+1024]
                            nc.tensor.matmul(acc[:, 0:512], SG[:], Vv[:, 0:512],
                                             start=False, stop=True)
                            nc.tensor.matmul(acc[:, 512:514], SG[:], Vv[:, 512:514],
                                             start=False, stop=True)
                        col = slice(kt * T, (kt + 1) * T)
                        accv = acc[:, 0:T]
                        if ch == 1:
                            nc.vector.tensor_copy(stC[:, col], accv)
                        else:
                            nc.scalar.mul(stN[:, col], accv, -1.0)
                            if kt < NA:
                                nc.scalar.copy(stP[:, kt * T:(kt + 1) * T], accv)

                outR_dst = outR_d.ap().rearrange("(kt p) t -> p kt t", kt=NKT)
                outI_dst = outI_d.ap().rearrange("(kt p) t -> p kt t", kt=NKT)
                nc.sync.dma_start(outR_dst, stC[:].rearrange("p (kt t) -> p kt t", kt=NKT))
                nc.sync.dma_start(outI_dst, stN[:].rearrange("p (kt t) -> p kt t", kt=NKT))
                for kt in range(NA):
                    p0 = 1 if kt == 0 else 0
                    s0 = kt * 128 + p0
                    cnt = (128 - p0) if kt < 7 else 1
                    nc.sync.dma_start(outIm_d.ap()[s0 - 1:s0 - 1 + cnt, :],
                                      stP[p0:p0 + cnt, kt * T:(kt + 1) * T])
    nc.compile()
    return nc


def host_prep(x, wsin, wcos):
    """Marshal full inputs into per-core input maps (pure data movement)."""
    x = np.asarray(x, dtype=np.float32)
    B = x.shape[0]
    xp = np.pad(x, ((0, 0), (1024, 1024)), mode="reflect")
    # layout A: XA[p, j] = xp[128j + p]
    XA = np.ascontiguousarray(xp.reshape(B, 2064, 128).transpose(0, 2, 1))
    # reversed layout: XR[p, j] = xp[128(j+1) - p] (out-of-range -> 0, unused)
    j = np.arange(2068); p = np.arange(128)
    idx = 128 * (j[None, :] + 1) - p[:, None]
    oob = idx >= L_PAD
    idx = np.where(oob, 0, idx)
    XR = xp[:, idx]
    XR[:, oob] = 0.0

    def phase_split(M, pitch=PH):
        B_, P_, C = M.shape
        out = np.zeros((B_, P_, 4, pitch), dtype=np.float32)
        for ph in range(4):
            col = M[:, :, ph::4]
            out[:, :, ph, :col.shape[2]] = col
        return np.ascontiguousarray(out.reshape(B_, P_, 4 * pitch))

    XA4 = phase_split(XA)
    XR4 = phase_split(XR)
    WT = np.concatenate([np.asarray(wsin).T[:1024, :NKT * 128],
                         np.asarray(wcos).T[:1024, :NKT * 128]],
                        axis=1).astype(np.float16)          # (1024, 2304)
    # block by k-tile so each 512KB block is one DMA and group kt only
    # depends on its own block: WH[kt*128+p, a*256+ch*128+c]
    WH = WT.reshape(NA, 128, 2, NKT, 128).transpose(3, 1, 0, 2, 4)
    WT = np.ascontiguousarray(WH.reshape(NKT * 128, 2048))
    sgn = np.zeros((128, 128), dtype=np.float16)
    sgn[0, :] = (-1.0) ** np.arange(128)
    return [{"xa": XA4[b], "xr": XR4[b], "wt": WT, "sgn": sgn} for b in range(B)]


def assemble(results):
    """Gather per-core outputs into the full (real, -imag) pair."""
    B = len(results)
    R = np.empty((B, 2048, T), np.float32)
    I = np.empty((B, 2048, T), np.float32)
    for b in range(B):
        r = results[b]
        R[b, :1152] = r["outR"]
        R[b, 1152:] = r["outR"][896:0:-1]     # cos mirror: copy
        I[b, :1152] = r["outI"]
        I[b, 1152:] = r["outIm"][::-1]        # sin mirror: +imag rows
    return R, I


class _Runner:
    """Build once, jit once, run many (shard_map over the 8 cores)."""

    def __init__(self, reps=1):
        import jax
        from jax.sharding import Mesh, PartitionSpec
        from jax.experimental.shard_map import shard_map
        from concourse.bass2jax import _bass_exec_p, install_neuronx_cc_hook

        install_neuronx_cc_hook()
        self.jax = jax
        nc = build_nc(reps=reps)
        self.nc = nc
        in_names, out_names, out_avals = [], [], []
        for alloc in nc.m.functions[0].allocations:
            if not isinstance(alloc, mybir.MemoryLocationSet):
                continue
            name = alloc.memorylocations[0].name
            if alloc.kind == "ExternalInput":
                in_names.append(name)
            elif alloc.kind == "ExternalOutput":
                out_names.append(name)
                out_avals.append(jax.core.ShapedArray(
                    tuple(alloc.tensor_shape), mybir.dt.np(alloc.dtype)))
        self.in_names, self.out_names, self.out_avals = in_names, out_names, out_avals
        n_params = len(in_names)
        all_names = in_names + out_names

        def _body(*args):
            outs = _bass_exec_p.bind(
                *args,
                out_avals=tuple(out_avals),
                in_names=tuple(all_names),
                out_names=tuple(out_names),
                lowering_input_output_aliases=(),
                sim_require_finite=True,
                sim_require_nnan=True,
                nc=nc,
            )
            return tuple(outs)

        devices = jax.devices()[:N_CORES]
        mesh = Mesh(np.asarray(devices), ("core",))
        n_outs = len(out_names)
        self._fn = jax.jit(
            shard_map(_body, mesh=mesh,
                      in_specs=(PartitionSpec("core"),) * (n_params + n_outs),
                      out_specs=(PartitionSpec("core"),) * n_outs,
                      check_rep=False),
            keep_unused=True,
        )
        self._zeros = [np.zeros((N_CORES * a.shape[0], *a.shape[1:]), a.dtype)
                       for a in out_avals]

    def prepare(self, in_maps):
        pid = self.nc.partition_id_tensor.name if self.nc.partition_id_tensor else None
        in_maps = [
            dict(m, **({pid: np.array([[c]], dtype=np.uint32)} if pid else {}))
            for c, m in enumerate(in_maps)
        ]
        concat = [np.concatenate([np.asarray(m[name]) for m in in_maps], axis=0)
                  for name in self.in_names]
        self._args = [self.jax.device_put(a) for a in concat + self._zeros]
        self.jax.block_until_ready(self._args)

    def run(self):
        out = self._fn(*self._args)
        self.jax.block_until_ready(out)
        return out

    def results(self, out):
        res = []
        for c in range(N_CORES):
            d = {}
            for i, name in enumerate(self.out_names):
                a = np.asarray(out[i])
                d[name] = a.reshape(N_CORES, *self.out_avals[i].shape)[c]
            res.append(d)
        return res


_RUNNER = None


def kernel(x, wsin, wcos):
    """Full inputs in, full output out: returns (real, -imag) as in reference."""
    global _RUNNER
    if _RUNNER is None:
        _RUNNER = _Runner(reps=1)
    ins = host_prep(x, wsin, wcos)
    _RUNNER.prepare(ins)
    out = _RUNNER.run()
    R, I = assemble(_RUNNER.results(out))
    return R, I

